# revision 1
# baseline (speedup 1.0000x reference)
"""Fused attention block (QKV conv -> 4-head attention -> proj -> BatchNorm -> LeakyReLU)
distributed over 8 trn2 NeuronCores, data-parallel over the batch dim.

Self-contained: hardcodes shapes B=8, C=64, N=2048, H=4.

Per-core layout tricks:
  - scores computed transposed (S^T = K^T Q, keys on partitions) so softmax
    normalization needs no transposes;
  - softmax denominators come free from a ones-column folded into the PV
    matmul's stationary operand;
  - normalization deferred to a single broadcast-multiply at the end;
  - BatchNorm stats all-reduced across cores ([128,2] f32), folded+replicated
    via a DRAM bounce, inv_std = exp(-0.5*ln(var+eps)).
"""
import numpy as np
import ml_dtypes

import concourse.bass as bass
import concourse.mybir as mybir
from concourse import bacc, tile
from concourse.bass_utils import run_bass_kernel_spmd

B, C, N, H, D = 8, 64, 2048, 4, 16
C2 = 2 * C           # 128 input channels after concat
NC = 1024            # query-dim chunk
NCH = N // NC        # 2 chunks
MT = N // 128        # 16 key tiles of 128
F32 = mybir.dt.float32
BF16 = mybir.dt.bfloat16
SCALE = float(D) ** -0.5
BN_EPS = 1e-5
LEAK = 0.2
N_CORES = 8
CNT = float(B * N)   # batchnorm population count

Alu = mybir.AluOpType
Act = mybir.ActivationFunctionType


def build():
    nc = bacc.Bacc("TRN2", target_bir_lowering=False, debug=False,
                   num_devices=N_CORES)
    x_p = nc.declare_dram_parameter("x", [C2, N], BF16, isOutput=False)
    wq_p = nc.declare_dram_parameter("wq", [C2, 128], BF16, isOutput=False)
    wk_p = nc.declare_dram_parameter("wk", [C2, 128], BF16, isOutput=False)
    wv_p = nc.declare_dram_parameter("wv", [C2, C], BF16, isOutput=False)
    wp_p = nc.declare_dram_parameter("wp", [C2, C], BF16, isOutput=False)
    g_p = nc.declare_dram_parameter("gamma", [C, 1], F32, isOutput=False)
    b_p = nc.declare_dram_parameter("beta", [C, 1], F32, isOutput=False)
    out_p = nc.declare_dram_parameter("out", [C, N], F32, isOutput=True)

    with tile.TileContext(nc) as tc:
        with (
            tc.tile_pool(name="sb", bufs=1) as sb,
            tc.tile_pool(name="ps_s", bufs=2, space="PSUM") as ps_s,
            tc.tile_pool(name="ps_pv", bufs=2, space="PSUM") as ps_pv,
            tc.tile_pool(name="pp", bufs=3) as pp,
            tc.tile_pool(name="ep", bufs=2) as ep,
            tc.tile_pool(name="dram", bufs=2, space="DRAM") as dram,
        ):
            # ---- persistent SBUF tiles
            x_sb = sb.tile([C2, N], BF16, tag="x")
            wq_sb = sb.tile([C2, 128], BF16, tag="wq")
            wk_sb = sb.tile([C2, 128], BF16, tag="wk")
            wv_sb = sb.tile([C2, C], BF16, tag="wv")
            wp_sb = sb.tile([C2, C], BF16, tag="wp")
            g_sb = sb.tile([C2, 1], F32, tag="g")     # gamma replicated x2
            b_sb = sb.tile([C2, 1], F32, tag="b")
            q_sb = sb.tile([C2, N], BF16, tag="q")    # head h rows 32h..32h+16
            k_sb = sb.tile([C2, N], BF16, tag="k")
            # per key-tile, per head: 32 cols = [16 V^T | 1 ones | 15 zeros]
            vt_sb = sb.tile([C2, MT * 128], BF16, tag="vt")
            y_sb = sb.tile([C2, NC], F32, tag="y")    # proj out, fold layout
            stats = sb.tile([C2, 2], F32, tag="stats")

            # spread prologue loads across engines' DMA queues
            nc.sync.dma_start(x_sb[:, 0:NC], x_p[:, 0:NC])
            nc.gpsimd.dma_start(wq_sb[:], wq_p[:])
            nc.gpsimd.dma_start(wk_sb[:], wk_p[:])
            nc.scalar.dma_start(x_sb[:, NC:N], x_p[:, NC:N])
            nc.sync.dma_start(wv_sb[:], wv_p[:])
            nc.sync.dma_start(wp_sb[:], wp_p[:])
            nc.gpsimd.dma_start(g_sb[0:C, :], g_p[:])
            nc.gpsimd.dma_start(g_sb[C:C2, :], g_p[:])
            nc.scalar.dma_start(b_sb[0:C, :], b_p[:])
            nc.scalar.dma_start(b_sb[C:C2, :], b_p[:])

            # ---- QKV projections (heads padded to 32-partition groups).
            # Chunk-0 Q/K first so attention can start early; evacuate per
            # 512-col piece so the first score matmul isn't gated on a full
            # 1024-col copy.
            def qk_chunk(dst, w, c):
                qp = ps_s.tile([C2, NC], F32, tag="s")
                for j in range(NC // 512):
                    nc.tensor.matmul(
                        qp[:, 512 * j:512 * (j + 1)], lhsT=w[:],
                        rhs=x_sb[:, NC * c + 512 * j: NC * c + 512 * (j + 1)])
                    nc.vector.tensor_copy(
                        dst[:, NC * c + 512 * j: NC * c + 512 * (j + 1)],
                        qp[:, 512 * j:512 * (j + 1)])

            qk_chunk(q_sb, wq_sb, 0)
            qk_chunk(k_sb, wk_sb, 0)
            qk_chunk(q_sb, wq_sb, 1)
            qk_chunk(k_sb, wk_sb, 1)

            # V^T zero fill + ones columns on the otherwise-idle gpsimd
            nc.gpsimd.memset(vt_sb[:], 0.0)
            ones_ap = vt_sb[:].rearrange(
                "q (p h e) -> q p h e", p=MT, h=H, e=32)[:, :, :, 16:17]
            nc.gpsimd.memset(ones_ap, 1.0)

            # all 16 V^T key tiles in ONE psum allocation (keeps the S-pool
            # slot rotation undisturbed during attention)
            vp_all = ps_s.tile([C2, MT * C], F32, tag="s")
            for p in range(MT):
                nc.tensor.matmul(vp_all[:, C * p:C * (p + 1)],
                                 lhsT=x_sb[:, 128 * p:128 * (p + 1)],
                                 rhs=wv_sb[:])
            vt_dst = vt_sb[:].rearrange(
                "q (p h e) -> q p h e", p=MT, h=H, e=32)[:, :, :, 0:16]
            vt_src = vp_all[:].rearrange(
                "q (p h d) -> q p h d", p=MT, h=H, d=D)
            nc.vector.tensor_copy(vt_dst, vt_src)

            def epilogue(c, pv):
                """Normalize chunk-c attention output, project, evac + stats."""
                recip = ep.tile([C2, NC], F32, tag="recip")
                nc.vector.reciprocal(recip[:], pv[:])
                rbc = ep.tile([C2, NC], F32, tag="rbc")
                rec_d = dram.tile([H, NC], F32, tag="rec_d")
                for h in range(H):
                    nc.sync.dma_start(rec_d[h:h + 1, :],
                                      recip[32 * h + 16:32 * h + 17, :])
                for h in range(H):
                    nc.sync.dma_start(
                        rbc[32 * h:32 * h + 32, :],
                        rec_d[h:h + 1, :].partition_broadcast(32))
                on = ep.tile([C2, NC], BF16, tag="on")
                nc.vector.tensor_mul(on[:], pv[:], rbc[:])
                # projection reuses the freed pv slot; chunk c on partitions 64c..
                yp = ps_pv.tile([C2, NC], F32, tag="pv")
                r = slice(64 * c, 64 * c + 64)
                for j in range(NC // 512):
                    nc.tensor.matmul(yp[r, 512 * j:512 * (j + 1)],
                                     lhsT=wp_sb[:], rhs=on[:, 512 * j:512 * (j + 1)],
                                     tile_position=(0, 64 * c))
                # evac + batchnorm partial stats for this chunk
                nc.vector.tensor_scalar(y_sb[r, :], yp[r, :], 1.0, 0.0,
                                        op0=Alu.mult, op1=Alu.add,
                                        accum_out=stats[r, 0:1])
                ysq = ep.tile([C2, NC], F32, tag="recip")
                nc.vector.scalar_tensor_tensor(ysq[r, :], y_sb[r, :], 0.0,
                                               y_sb[r, :], op0=Alu.add,
                                               op1=Alu.mult,
                                               accum_out=stats[r, 1:2])

            # ---- attention, one query chunk at a time; previous chunk's
            # epilogue is emitted a few tiles into the next chunk so the PE
            # instruction stream never stalls waiting for it.
            prev = None
            for c in range(NCH):
                pv = ps_pv.tile([C2, NC], F32, tag="pv")
                pend = None

                def flush(pv=pv, c=c):
                    """exp + PV for the S tile emitted one pair ago. Keeping a
                    one-pair lag means the PE stream interleaves as
                    [S(k+1), PV(k)], so the activation engine never waits on a
                    score matmul stuck behind a PV that waits on it."""
                    nonlocal pend
                    if pend is None:
                        return
                    sp, t, h = pend
                    pend = None
                    pt = pp.tile([C2, NC], BF16, tag="p")
                    nc.scalar.activation(pt[:], sp[:], Act.Exp, scale=SCALE)
                    for j in range(NC // 512):
                        nc.tensor.matmul(
                            pv[32 * h:32 * h + 32, 512 * j:512 * (j + 1)],
                            lhsT=vt_sb[:, 128 * t + 32 * h: 128 * t + 32 * h + 32],
                            rhs=pt[:, 512 * j:512 * (j + 1)],
                            start=(t == 0), stop=(t == MT - 1),
                            skip_group_check=True,
                            tile_position=(0, 32 * h))

                for t in range(MT):
                    if c == 1 and t == 4 and prev is not None:
                        epilogue(*prev)
                        prev = None
                    for h in range(H):
                        sp = ps_s.tile([C2, NC], F32, tag="s")
                        for j in range(NC // 512):
                            nc.tensor.matmul(
                                sp[:, 512 * j:512 * (j + 1)],
                                lhsT=k_sb[32 * h:32 * h + 16, 128 * t:128 * (t + 1)],
                                rhs=q_sb[32 * h:32 * h + 16,
                                         NC * c + 512 * j: NC * c + 512 * (j + 1)],
                                tile_position=(32 * h, 0))
                        flush()
                        pend = (sp, t, h)
                flush()
                prev = (c, pv)
            epilogue(*prev)

            # ---- cross-core reduce of batchnorm stats [128, 2]
            st_in = dram.tile([C2, 2], F32, tag="st_in")
            st_out = dram.tile([C2, 2], F32, tag="st_out")
            nc.gpsimd.dma_start(st_in[:], stats[:])
            nc.gpsimd.collective_compute(
                "AllReduce", Alu.add,
                replica_groups=[list(range(N_CORES))],
                ins=[st_in.opt()], outs=[st_out.opt()])
            # fold the two channel halves and replicate to all 128 partitions
            fa = sb.tile([C2, 2], F32, tag="fa")
            fb = sb.tile([C2, 2], F32, tag="fb")
            nc.sync.dma_start(fa[:], st_out[:])
            nc.sync.dma_start(fb[0:C, :], st_out[C:C2, :])
            nc.sync.dma_start(fb[C:C2, :], st_out[0:C, :])
            red = sb.tile([C2, 2], F32, tag="red")
            nc.vector.tensor_add(red[:], fa[:], fb[:])

            # ---- finalize: mean/var -> scale/shift, all [128, 1] replicated
            mean = sb.tile([C2, 1], F32, tag="mean")
            ex2 = sb.tile([C2, 1], F32, tag="ex2")
            nc.vector.tensor_scalar_mul(mean[:], red[:, 0:1], 1.0 / CNT)
            nc.vector.tensor_scalar_mul(ex2[:], red[:, 1:2], 1.0 / CNT)
            msq = sb.tile([C2, 1], F32, tag="msq")
            nc.vector.tensor_mul(msq[:], mean[:], mean[:])
            var = sb.tile([C2, 1], F32, tag="var")
            nc.vector.tensor_sub(var[:], ex2[:], msq[:])
            eps_t = sb.tile([C2, 1], F32, tag="eps")
            nc.vector.memset(eps_t[:], BN_EPS)
            lnv = sb.tile([C2, 1], F32, tag="lnv")
            nc.scalar.activation(lnv[:], var[:], Act.Ln, bias=eps_t[:, 0:1])
            istd = sb.tile([C2, 1], F32, tag="istd")
            nc.scalar.activation(istd[:], lnv[:], Act.Exp, scale=-0.5)
            sc = sb.tile([C2, 1], F32, tag="sc")
            nc.vector.tensor_mul(sc[:], g_sb[:], istd[:])
            msc = sb.tile([C2, 1], F32, tag="msc")
            nc.vector.tensor_scalar(msc[:], mean[:], sc[:, 0:1], None, op0=Alu.mult)
            sh = sb.tile([C2, 1], F32, tag="sh")
            nc.vector.tensor_sub(sh[:], b_sb[:], msc[:])

            # ---- normalize + LeakyReLU + store
            yn = ep.tile([C2, NC], F32, tag="rbc")
            nc.vector.tensor_scalar(yn[:], y_sb[:], sc[:, 0:1], sh[:, 0:1],
                                    op0=Alu.mult, op1=Alu.add)
            yl = ep.tile([C2, NC], F32, tag="recip")
            nc.vector.scalar_tensor_tensor(yl[:], yn[:], LEAK, yn[:],
                                           op0=Alu.mult, op1=Alu.max)
            nc.sync.dma_start(out_p[:, 0:NC], yl[0:C, :])
            nc.sync.dma_start(out_p[:, NC:N], yl[C:C2, :])

    nc.compile()

    # Post-compile surgery: Exp..Ln..Exp makes the table-load inserter
    # ping-pong between exp_and_others and the ln set (3 loads). One set
    # covers every activation used here; point the first load at it and
    # drop the rest. Loads are inserted after semaphore generation, so
    # they carry no sync state and removal is safe.
    from concourse.hw_specs import get_activation_tables
    tabs = list(get_activation_tables(nc.m.arch).keys())
    nle = tabs.index("natural_log_exp_and_others")
    loads = [(b, i) for b in nc.main_func.blocks for i in b.instructions
             if isinstance(i, mybir.InstLoadActFuncSet)]
    if loads:
        loads[0][1].act_func_set_id = nle
        for b, i in loads[1:]:
            b.instructions.remove(i)
    return nc


_NC_CACHE = None


def _get_nc():
    global _NC_CACHE
    if _NC_CACHE is None:
        _NC_CACHE = build()
    return _NC_CACHE


def _prep_inputs(x_local, x_branch, w_qkv, w_proj, gamma, beta):
    bf16 = ml_dtypes.bfloat16
    x_local = np.asarray(x_local, np.float32)
    x_branch = np.asarray(x_branch, np.float32)
    w_qkv = np.asarray(w_qkv, np.float32)
    w_proj = np.asarray(w_proj, np.float32)
    gamma = np.asarray(gamma, np.float32)
    beta = np.asarray(beta, np.float32)

    X = np.concatenate([x_local, x_branch], axis=1).astype(bf16)  # [B, 128, N]
    WT = w_qkv.T.copy()  # [128, 192]
    wq = np.zeros((C2, 128), np.float32)
    wk = np.zeros((C2, 128), np.float32)
    for h in range(H):
        wq[:, 32 * h:32 * h + D] = WT[:, D * h:D * (h + 1)]
        wk[:, 32 * h:32 * h + D] = WT[:, C + D * h:C + D * (h + 1)]
    wv = WT[:, 2 * C:3 * C]
    wp = np.zeros((C2, C), np.float32)
    for h in range(H):
        wp[32 * h:32 * h + D, :] = w_proj[:, D * h:D * (h + 1)].T
    common = dict(
        wq=wq.astype(bf16), wk=wk.astype(bf16), wv=np.ascontiguousarray(wv).astype(bf16),
        wp=wp.astype(bf16),
        gamma=np.ascontiguousarray(gamma.reshape(C, 1)),
        beta=np.ascontiguousarray(beta.reshape(C, 1)),
    )
    return [dict(x=np.ascontiguousarray(X[b]), **common) for b in range(B)]


def kernel(x_local, x_branch, w_qkv, w_proj, gamma, beta, _trace=False, _tmpdir=None):
    nc = _get_nc()
    in_maps = _prep_inputs(x_local, x_branch, w_qkv, w_proj, gamma, beta)
    res = run_bass_kernel_spmd(nc, in_maps, core_ids=list(range(N_CORES)),
                               trace=_trace, tmpdir=_tmpdir)
    out = np.stack([np.asarray(res.results[i]["out"]) for i in range(N_CORES)])
    if _trace:
        kernel._last_results = res
    return out.astype(np.float32)



# revision 10
# speedup vs baseline: 1.3912x; 1.3912x over previous
"""Fused attention block (QKV conv -> 4-head attention -> proj -> BatchNorm -> LeakyReLU)
distributed over 8 trn2 NeuronCores, data-parallel over the batch dim.

Self-contained: hardcodes shapes B=8, C=64, N=2048, H=4.

The kernel is exp-throughput bound (H*N*N = 16.8M exps/core), so the design
keeps both exp-capable engines (ScalarE ACT, VectorE DVE) saturated and hides
everything else under them:
  - scores computed transposed (S^T = K^T Q, keys on partitions) in
    [128 x 512] PSUM tiles through a 5-slot ring; the 4 heads' score matmuls
    issue back-to-back into distinct PE row-groups -> 4x tile concurrency;
  - exp split by head with per-softmax-row purity: heads 0,2 exact exp on
    ScalarE; heads 1,3 on VectorE via a bf16-Schraudolph bit trick
    (int16(FA*s + FB) reinterpreted as bf16 ~= exp(s*scale)); the ~3%
    sawtooth is common to numerator and denominator of those rows and
    largely cancels;
  - PV matmuls col-group tiled (4x concurrent), softmax denominators come
    free from a ones-column folded into the stationary V^T operand;
  - per-query normalization deferred: 1/denom = exp(-ln d) on ScalarE in a
    compact [128,32] layout (DRAM bounce for the transpose), then one
    broadcast-multiply;
  - BatchNorm stats all-reduced across cores ([128,4] f32), folded+replicated
    via a DRAM bounce, inv_std = exp(-0.5*ln(var+eps)); cheap SBUF-only
    elementwise tail ops run on the otherwise idle GpSimd.
"""
import numpy as np
import ml_dtypes

import concourse.bass as bass
import concourse.mybir as mybir
from concourse import bacc, tile
from concourse.bass_utils import run_bass_kernel_spmd

B, C, N, H, D = 8, 64, 2048, 4, 16
C2 = 2 * C           # 128 input channels after concat
NC = 1024            # query-dim chunk
NCH = N // NC        # 2 chunks
MT = N // 128        # 16 key tiles of 128
F32 = mybir.dt.float32
BF16 = mybir.dt.bfloat16
I16 = mybir.dt.int16
SCALE = float(D) ** -0.5
BN_EPS = 1e-5
LEAK = 0.2
N_CORES = 8
CNT = float(B * N)   # batchnorm population count

# bf16-Schraudolph: bf16 bits of exp(s*SCALE) ~= int16(FA*s + FB)
_LN2 = float(np.log(2.0))
FA = 128.0 / _LN2 * SCALE
FB = 127.0 * 128.0 - 4.75

ACT_HEADS = (0, 2)   # exact exp on ScalarE
                     # heads 1,3: fast exp on VectorE

Alu = mybir.AluOpType
Act = mybir.ActivationFunctionType


def build():
    nc = bacc.Bacc("TRN2", target_bir_lowering=False, debug=False,
                   num_devices=N_CORES)
    x_p = nc.declare_dram_parameter("x", [C2, N], BF16, isOutput=False)
    wq_p = nc.declare_dram_parameter("wq", [C2, 128], BF16, isOutput=False)
    wk_p = nc.declare_dram_parameter("wk", [C2, 128], BF16, isOutput=False)
    wv_p = nc.declare_dram_parameter("wv", [C2, C], BF16, isOutput=False)
    wp_p = nc.declare_dram_parameter("wp", [C2, C], BF16, isOutput=False)
    g_p = nc.declare_dram_parameter("gamma", [C, 1], F32, isOutput=False)
    b_p = nc.declare_dram_parameter("beta", [C, 1], F32, isOutput=False)
    out_p = nc.declare_dram_parameter("out", [C, N], F32, isOutput=True)

    with tile.TileContext(nc) as tc:
        with (
            tc.tile_pool(name="sb", bufs=1) as sb,
            tc.tile_pool(name="ps_s", bufs=5, space="PSUM") as ps_s,
            tc.tile_pool(name="ps_pv", bufs=1, space="PSUM") as ps_pv,
            tc.tile_pool(name="pp", bufs=3) as pp,
            tc.tile_pool(name="fp", bufs=3) as fp,
            tc.tile_pool(name="ep", bufs=2) as ep,
            tc.tile_pool(name="dram", bufs=2, space="DRAM") as dram,
        ):
            # ---- persistent SBUF tiles
            x_sb = sb.tile([C2, N], BF16, tag="x")
            wq_sb = sb.tile([C2, 128], BF16, tag="wq")
            wk_sb = sb.tile([C2, 128], BF16, tag="wk")
            wv_sb = sb.tile([C2, C], BF16, tag="wv")
            wp_sb = sb.tile([C2, C], BF16, tag="wp")
            g_sb = sb.tile([C2, 1], F32, tag="g")     # gamma replicated x2
            b_sb = sb.tile([C2, 1], F32, tag="b")
            q_sb = sb.tile([C2, N], BF16, tag="q")    # head h rows 32h..32h+16
            k_sb = sb.tile([C2, N], BF16, tag="k")
            # per key-tile, per head: 32 cols = [16 V^T | 1 ones | 15 zeros]
            vt_sb = sb.tile([C2, MT * 128], BF16, tag="vt")
            y_sb = sb.tile([C2, NC], F32, tag="y")    # proj out, fold layout
            stats = sb.tile([C2, 4], F32, tag="stats")  # sum_j0 sum_j1 sq_j0 sq_j1

            # spread prologue loads across engines' DMA queues
            nc.sync.dma_start(x_sb[:, 0:NC], x_p[:, 0:NC])
            nc.gpsimd.dma_start(wq_sb[:], wq_p[:])
            nc.gpsimd.dma_start(wk_sb[:], wk_p[:])
            nc.scalar.dma_start(x_sb[:, NC:N], x_p[:, NC:N])
            nc.sync.dma_start(wv_sb[:], wv_p[:])
            nc.sync.dma_start(wp_sb[:], wp_p[:])
            nc.gpsimd.dma_start(g_sb[0:C, :], g_p[:])
            nc.gpsimd.dma_start(g_sb[C:C2, :], g_p[:])
            nc.scalar.dma_start(b_sb[0:C, :], b_p[:])
            nc.scalar.dma_start(b_sb[C:C2, :], b_p[:])

            # ---- QKV projections (heads padded to 32-partition groups).
            # Evacuate per 512-col piece, alternating DVE/ACT so the first
            # score matmul isn't gated on one engine.
            def qk_chunk(dst, w, c, eng_flip):
                for j in range(NC // 512):
                    qp = ps_s.tile([C2, 512], F32, tag="s", name="qp")
                    nc.tensor.matmul(
                        qp[:], lhsT=w[:],
                        rhs=x_sb[:, NC * c + 512 * j: NC * c + 512 * (j + 1)])
                    dslice = dst[:, NC * c + 512 * j: NC * c + 512 * (j + 1)]
                    if (j + eng_flip) % 2 == 0:
                        nc.vector.tensor_copy(dslice, qp[:])
                    else:
                        nc.scalar.copy(dslice, qp[:])

            qk_chunk(q_sb, wq_sb, 0, 0)
            qk_chunk(k_sb, wk_sb, 0, 1)
            qk_chunk(q_sb, wq_sb, 1, 0)
            qk_chunk(k_sb, wk_sb, 1, 1)

            # V^T zero fill + ones columns on the otherwise-idle gpsimd
            nc.gpsimd.memset(vt_sb[:], 0.0)
            ones_ap = vt_sb[:].rearrange(
                "q (p h e) -> q p h e", p=MT, h=H, e=32)[:, :, :, 16:17]
            nc.gpsimd.memset(ones_ap, 1.0)

            # V^T: 16 key tiles, two 512-col psum ring slots of 8 tiles each
            for half in range(2):
                vp = ps_s.tile([C2, 512], F32, tag="s", name="vp")
                for p in range(8):
                    pk = 8 * half + p
                    nc.tensor.matmul(vp[:, C * p:C * (p + 1)],
                                     lhsT=x_sb[:, 128 * pk:128 * (pk + 1)],
                                     rhs=wv_sb[:])
                vt_dst = vt_sb[:, 1024 * half:1024 * (half + 1)].rearrange(
                    "q (p h e) -> q p h e", p=8, h=H, e=32)[:, :, :, 0:16]
                vt_src = vp[:].rearrange(
                    "q (p h d) -> q p h d", p=8, h=H, d=D)
                nc.vector.tensor_copy(vt_dst, vt_src)

            def epilogue(c, pv):
                """Normalize chunk-c attention output, project, evac + stats."""
                # evacuate pv to SBUF (frees the psum slot for the next chunk)
                pvs = ep.tile([C2, NC], F32, tag="pvs")
                nc.vector.tensor_copy(pvs[:], pv[:])
                # denominators pvs[32h+16, :] -> DRAM [4, NC] -> compact
                # [128, 32]; 1/d = exp(-ln d) on ScalarE; back out + bcast
                den_d = dram.tile([H, NC], F32, tag="den_d")
                for h in range(H):
                    nc.sync.dma_start(den_d[h:h + 1, :],
                                      pvs[32 * h + 16:32 * h + 17, :])
                den_c = ep.tile([C2, NC // 32], F32, tag="den_c")
                nc.sync.dma_start(
                    den_c[:],
                    den_d[:].rearrange("h (p q) -> (h p) q", p=32, q=32))
                lnd = ep.tile([C2, NC // 32], F32, tag="lnd")
                nc.scalar.activation(lnd[:], den_c[:], Act.Ln)
                rcp = ep.tile([C2, NC // 32], F32, tag="rcp")
                nc.scalar.activation(rcp[:], lnd[:], Act.Exp, scale=-1.0)
                rec_d = dram.tile([H, NC], F32, tag="rec_d")
                nc.sync.dma_start(
                    rec_d[:].rearrange("h (p q) -> (h p) q", p=32, q=32),
                    rcp[:])
                rbc = ep.tile([C2, NC], F32, tag="rbc")
                for h in range(H):
                    nc.sync.dma_start(
                        rbc[32 * h:32 * h + 32, :],
                        rec_d[h:h + 1, :].partition_broadcast(32))
                on = ep.tile([C2, NC], BF16, tag="on")
                nc.vector.tensor_mul(on[:], pvs[:], rbc[:])
                # projection into s-ring slots; chunk c rows 64c..64c+64
                r = slice(64 * c, 64 * c + 64)
                for j in range(NC // 512):
                    yp = ps_s.tile([C2, 512], F32, tag="s", name="yp")
                    nc.tensor.matmul(yp[r, :],
                                     lhsT=wp_sb[:], rhs=on[:, 512 * j:512 * (j + 1)],
                                     tile_position=(0, 64 * c))
                    nc.vector.tensor_scalar(y_sb[r, 512 * j:512 * (j + 1)],
                                            yp[r, :], 1.0, 0.0,
                                            op0=Alu.mult, op1=Alu.add,
                                            accum_out=stats[r, j:j + 1])
                    ysq = ep.tile([C2, 512], F32, tag="ysq")
                    nc.vector.scalar_tensor_tensor(
                        ysq[r, :], y_sb[r, 512 * j:512 * (j + 1)], 0.0,
                        y_sb[r, 512 * j:512 * (j + 1)], op0=Alu.add,
                        op1=Alu.mult, accum_out=stats[r, 2 + j:3 + j])

            # ---- attention: per (chunk, key-tile, j-half): 4 S matmuls into
            # the 5-deep psum ring (distinct PE row-groups -> concurrent),
            # exp on ScalarE (heads 0,2) / VectorE (heads 1,3 bit-trick),
            # then 4 col-tiled PV matmuls accumulate into the pv slot.  The
            # PV group lags one (t,j) step so the PE emits the NEXT step's
            # score matmuls before the PV that waits on this step's exps —
            # the exp engines then never wait on a fresh score tile.
            for c in range(NCH):
                pv = ps_pv.tile([C2, NC], F32, tag="pv", name="pv")
                prev_pts = None

                def flush_pv(pv=pv):
                    nonlocal prev_pts
                    if prev_pts is None:
                        return
                    for pt, t, j, h in prev_pts:
                        nc.tensor.matmul(
                            pv[32 * h:32 * h + 32, 512 * j:512 * (j + 1)],
                            lhsT=vt_sb[:, 128 * t + 32 * h: 128 * t + 32 * h + 32],
                            rhs=pt[:],
                            start=(t == 0), stop=(t == MT - 1),
                            skip_group_check=True,
                            tile_position=(0, 32 * h))
                    prev_pts = None

                for t in range(MT):
                    for j in range(NC // 512):
                        sps = []
                        for h in range(H):
                            sp = ps_s.tile([C2, 512], F32, tag="s", name="sp")
                            nc.tensor.matmul(
                                sp[:],
                                lhsT=k_sb[32 * h:32 * h + 16, 128 * t:128 * (t + 1)],
                                rhs=q_sb[32 * h:32 * h + 16,
                                         NC * c + 512 * j: NC * c + 512 * (j + 1)],
                                tile_position=(32 * h, 0))
                            sps.append((sp, h))
                        pts = []
                        for sp, h in sps:
                            if h in ACT_HEADS:
                                pt = pp.tile([C2, 512], BF16, tag="p")
                                nc.scalar.activation(pt[:], sp[:], Act.Exp,
                                                     scale=SCALE)
                                pts.append((pt[:], t, j, h))
                            else:
                                pti = fp.tile([C2, 512], I16, tag="pf")
                                nc.vector.tensor_scalar(pti[:], sp[:], FA, FB,
                                                        op0=Alu.mult,
                                                        op1=Alu.add)
                                pts.append((pti[:].bitcast(BF16), t, j, h))
                        flush_pv()
                        prev_pts = pts
                flush_pv()
                epilogue(c, pv)

            # ---- cross-core reduce of batchnorm stats [128, 4]
            st_in = dram.tile([C2, 4], F32, tag="st_in")
            st_out = dram.tile([C2, 4], F32, tag="st_out")
            nc.gpsimd.dma_start(st_in[:], stats[:])
            nc.gpsimd.collective_compute(
                "AllReduce", Alu.add,
                replica_groups=[list(range(N_CORES))],
                ins=[st_in.opt()], outs=[st_out.opt()])
            # fold the two channel halves and replicate to all 128 partitions
            fa = sb.tile([C2, 4], F32, tag="fa")
            fb = sb.tile([C2, 4], F32, tag="fb")
            nc.sync.dma_start(fa[:], st_out[:])
            nc.sync.dma_start(fb[0:C, :], st_out[C:C2, :])
            nc.sync.dma_start(fb[C:C2, :], st_out[0:C, :])
            red = sb.tile([C2, 4], F32, tag="red")
            nc.gpsimd.tensor_add(red[:], fa[:], fb[:])

            # ---- finalize: mean/var -> scale/shift, all [128, 1] replicated
            # (cheap SBUF-only ops on gpsimd; ln/exp on ScalarE)
            s01 = sb.tile([C2, 1], F32, tag="s01")
            s23 = sb.tile([C2, 1], F32, tag="s23")
            nc.gpsimd.tensor_add(s01[:], red[:, 0:1], red[:, 1:2])
            nc.gpsimd.tensor_add(s23[:], red[:, 2:3], red[:, 3:4])
            mean = sb.tile([C2, 1], F32, tag="mean")
            ex2 = sb.tile([C2, 1], F32, tag="ex2")
            nc.gpsimd.tensor_scalar_mul(mean[:], s01[:], 1.0 / CNT)
            nc.gpsimd.tensor_scalar_mul(ex2[:], s23[:], 1.0 / CNT)
            msq = sb.tile([C2, 1], F32, tag="msq")
            nc.gpsimd.tensor_mul(msq[:], mean[:], mean[:])
            var = sb.tile([C2, 1], F32, tag="var")
            nc.gpsimd.tensor_sub(var[:], ex2[:], msq[:])
            eps_t = sb.tile([C2, 1], F32, tag="eps")
            nc.gpsimd.memset(eps_t[:], BN_EPS)
            lnv = sb.tile([C2, 1], F32, tag="lnv")
            nc.scalar.activation(lnv[:], var[:], Act.Ln, bias=eps_t[:, 0:1])
            istd = sb.tile([C2, 1], F32, tag="istd")
            nc.scalar.activation(istd[:], lnv[:], Act.Exp, scale=-0.5)
            sc = sb.tile([C2, 1], F32, tag="sc")
            nc.gpsimd.tensor_mul(sc[:], g_sb[:], istd[:])
            msc = sb.tile([C2, 1], F32, tag="msc")
            nc.vector.tensor_scalar(msc[:], mean[:], sc[:, 0:1], None, op0=Alu.mult)
            sh = sb.tile([C2, 1], F32, tag="sh")
            nc.gpsimd.tensor_sub(sh[:], b_sb[:], msc[:])

            # ---- normalize + LeakyReLU + store (gpsimd + vector split)
            yn = ep.tile([C2, NC], F32, tag="rbc")
            nc.vector.tensor_scalar(yn[:], y_sb[:],
                                    sc[:, 0:1], sh[:, 0:1],
                                    op0=Alu.mult, op1=Alu.add)
            yl = ep.tile([C2, NC], F32, tag="on2")
            nc.vector.scalar_tensor_tensor(yl[:], yn[:], LEAK, yn[:],
                                           op0=Alu.mult, op1=Alu.max)

            nc.sync.dma_start(out_p[:, 0:NC], yl[0:C, :])
            nc.sync.dma_start(out_p[:, NC:N], yl[C:C2, :])

    nc.compile()

    # Post-compile surgery: one activation table set covers Exp+Ln; point the
    # first inserted load at it and drop the rest (loads are inserted after
    # semaphore generation, so they carry no sync state and removal is safe).
    from concourse.hw_specs import get_activation_tables
    tabs = list(get_activation_tables(nc.m.arch).keys())
    nle = tabs.index("natural_log_exp_and_others")
    loads = [(b, i) for b in nc.main_func.blocks for i in b.instructions
             if isinstance(i, mybir.InstLoadActFuncSet)]
    if loads:
        loads[0][1].act_func_set_id = nle
        for b, i in loads[1:]:
            b.instructions.remove(i)
    return nc


_NC_CACHE = None


def _get_nc():
    global _NC_CACHE
    if _NC_CACHE is None:
        _NC_CACHE = build()
    return _NC_CACHE


def _prep_inputs(x_local, x_branch, w_qkv, w_proj, gamma, beta):
    bf16 = ml_dtypes.bfloat16
    x_local = np.asarray(x_local, np.float32)
    x_branch = np.asarray(x_branch, np.float32)
    w_qkv = np.asarray(w_qkv, np.float32)
    w_proj = np.asarray(w_proj, np.float32)
    gamma = np.asarray(gamma, np.float32)
    beta = np.asarray(beta, np.float32)

    X = np.concatenate([x_local, x_branch], axis=1).astype(bf16)  # [B, 128, N]
    WT = w_qkv.T.copy()  # [128, 192]
    wq = np.zeros((C2, 128), np.float32)
    wk = np.zeros((C2, 128), np.float32)
    for h in range(H):
        wq[:, 32 * h:32 * h + D] = WT[:, D * h:D * (h + 1)]
        wk[:, 32 * h:32 * h + D] = WT[:, C + D * h:C + D * (h + 1)]
    wv = WT[:, 2 * C:3 * C]
    wp = np.zeros((C2, C), np.float32)
    for h in range(H):
        wp[32 * h:32 * h + D, :] = w_proj[:, D * h:D * (h + 1)].T
    common = dict(
        wq=wq.astype(bf16), wk=wk.astype(bf16), wv=np.ascontiguousarray(wv).astype(bf16),
        wp=wp.astype(bf16),
        gamma=np.ascontiguousarray(gamma.reshape(C, 1)),
        beta=np.ascontiguousarray(beta.reshape(C, 1)),
    )
    return [dict(x=np.ascontiguousarray(X[b]), **common) for b in range(B)]


def kernel(x_local, x_branch, w_qkv, w_proj, gamma, beta, _trace=False, _tmpdir=None):
    nc = _get_nc()
    in_maps = _prep_inputs(x_local, x_branch, w_qkv, w_proj, gamma, beta)
    res = run_bass_kernel_spmd(nc, in_maps, core_ids=list(range(N_CORES)),
                               trace=_trace, tmpdir=_tmpdir)
    out = np.stack([np.asarray(res.results[i]["out"]) for i in range(N_CORES)])
    if _trace:
        kernel._last_results = res
    return out.astype(np.float32)


# revision 12
# speedup vs baseline: 1.4620x; 1.0509x over previous
"""Fused attention block (QKV conv -> 4-head attention -> proj -> BatchNorm -> LeakyReLU)
distributed over 8 trn2 NeuronCores, data-parallel over the batch dim.

Self-contained: hardcodes shapes B=8, C=64, N=2048, H=4.

The kernel is exp-throughput bound (H*N*N = 16.8M exps/core), so the design
keeps both exp-capable engines (ScalarE ACT, VectorE DVE) saturated and hides
everything else under them:
  - scores computed transposed (S^T = K^T Q, keys on partitions) in
    [128 x 512] PSUM tiles through a 5-slot ring; the 4 heads' score matmuls
    issue back-to-back into distinct PE row-groups -> 4x tile concurrency;
  - exp split by head with per-softmax-row purity: heads 0,2 exact exp on
    ScalarE; heads 1,3 on VectorE via a bf16-Schraudolph bit trick
    (int16(FA*s + FB) reinterpreted as bf16 ~= exp(s*scale)); the ~3%
    sawtooth is common to numerator and denominator of those rows and
    largely cancels;
  - PV matmuls col-group tiled (4x concurrent), softmax denominators come
    free from a ones-column folded into the stationary V^T operand;
  - per-query normalization deferred: 1/denom = exp(-ln d) on ScalarE in a
    compact [128,32] layout (DRAM bounce for the transpose), then one
    broadcast-multiply;
  - BatchNorm stats all-reduced across cores ([128,4] f32), folded+replicated
    via a DRAM bounce, inv_std = exp(-0.5*ln(var+eps)); cheap SBUF-only
    elementwise tail ops run on the otherwise idle GpSimd.
"""
import numpy as np
import ml_dtypes

import concourse.bass as bass
import concourse.mybir as mybir
from concourse import bacc, tile
from concourse.bass_utils import run_bass_kernel_spmd

B, C, N, H, D = 8, 64, 2048, 4, 16
C2 = 2 * C           # 128 input channels after concat
NC = 1024            # query-dim chunk
NCH = N // NC        # 2 chunks
MT = N // 128        # 16 key tiles of 128
F32 = mybir.dt.float32
BF16 = mybir.dt.bfloat16
I16 = mybir.dt.int16
SCALE = float(D) ** -0.5
BN_EPS = 1e-5
LEAK = 0.2
N_CORES = 8
CNT = float(B * N)   # batchnorm population count

# bf16-Schraudolph: bf16 bits of exp(s*SCALE) ~= int16(FA*s + FB)
_LN2 = float(np.log(2.0))
FA = 128.0 / _LN2 * SCALE
FB = 127.0 * 128.0 - 4.75

ACT_HEADS = (0, 2)   # exact exp on ScalarE
                     # heads 1,3: fast exp on VectorE

Alu = mybir.AluOpType
Act = mybir.ActivationFunctionType


def build():
    nc = bacc.Bacc("TRN2", target_bir_lowering=False, debug=False,
                   num_devices=N_CORES)
    x_p = nc.declare_dram_parameter("x", [C2, N], BF16, isOutput=False)
    wq_p = nc.declare_dram_parameter("wq", [C2, 128], BF16, isOutput=False)
    wk_p = nc.declare_dram_parameter("wk", [C2, 128], BF16, isOutput=False)
    wv_p = nc.declare_dram_parameter("wv", [C2, C], BF16, isOutput=False)
    wp_p = nc.declare_dram_parameter("wp", [C2, C], BF16, isOutput=False)
    g_p = nc.declare_dram_parameter("gamma", [C, 1], F32, isOutput=False)
    b_p = nc.declare_dram_parameter("beta", [C, 1], F32, isOutput=False)
    out_p = nc.declare_dram_parameter("out", [C, N], F32, isOutput=True)

    with tile.TileContext(nc) as tc:
        with (
            tc.tile_pool(name="sb", bufs=1) as sb,
            tc.tile_pool(name="ps_a", bufs=3, space="PSUM") as ps_a,
            tc.tile_pool(name="ps_d", bufs=3, space="PSUM") as ps_d,
            tc.tile_pool(name="ps_pv", bufs=1, space="PSUM") as ps_pv,
            tc.tile_pool(name="pp", bufs=3) as pp,
            tc.tile_pool(name="fp", bufs=3) as fp,
            tc.tile_pool(name="ep", bufs=2) as ep,
            tc.tile_pool(name="dram", bufs=2, space="DRAM") as dram,
        ):
            # ---- persistent SBUF tiles
            x_sb = sb.tile([C2, N], BF16, tag="x")
            wq_sb = sb.tile([C2, 128], BF16, tag="wq")
            wk_sb = sb.tile([C2, 128], BF16, tag="wk")
            wv_sb = sb.tile([C2, C], BF16, tag="wv")
            wp_sb = sb.tile([C2, C], BF16, tag="wp")
            g_sb = sb.tile([C2, 1], F32, tag="g")     # gamma replicated x2
            b_sb = sb.tile([C2, 1], F32, tag="b")
            q_sb = sb.tile([C2, N], BF16, tag="q")    # head h rows 32h..32h+16
            k_sb = sb.tile([C2, N], BF16, tag="k")
            # per key-tile, per head: 32 cols = [16 V^T | 1 ones | 15 zeros]
            vt_sb = sb.tile([C2, MT * 128], BF16, tag="vt")
            y_sb = sb.tile([C2, NC], F32, tag="y")    # proj out, fold layout
            stats = sb.tile([C2, 4], F32, tag="stats")  # sum_j0 sum_j1 sq_j0 sq_j1

            # spread prologue loads across engines' DMA queues
            nc.sync.dma_start(x_sb[:, 0:NC], x_p[:, 0:NC])
            nc.gpsimd.dma_start(wq_sb[:], wq_p[:])
            nc.gpsimd.dma_start(wk_sb[:], wk_p[:])
            nc.scalar.dma_start(x_sb[:, NC:N], x_p[:, NC:N])
            nc.sync.dma_start(wv_sb[:], wv_p[:])
            nc.sync.dma_start(wp_sb[:], wp_p[:])
            nc.gpsimd.dma_start(g_sb[0:C, :], g_p[:])
            nc.gpsimd.dma_start(g_sb[C:C2, :], g_p[:])
            nc.scalar.dma_start(b_sb[0:C, :], b_p[:])
            nc.scalar.dma_start(b_sb[C:C2, :], b_p[:])

            # ---- QKV projections (heads padded to 32-partition groups).
            # Evacuate per 512-col piece, alternating DVE/ACT so the first
            # score matmul isn't gated on one engine.
            def qk_chunk(dst, w, c, eng_flip):
                for j in range(NC // 512):
                    qp = ps_a.tile([C2, 512], F32, tag="sa", name="qp")
                    nc.tensor.matmul(
                        qp[:], lhsT=w[:],
                        rhs=x_sb[:, NC * c + 512 * j: NC * c + 512 * (j + 1)])
                    dslice = dst[:, NC * c + 512 * j: NC * c + 512 * (j + 1)]
                    if (j + eng_flip) % 2 == 0:
                        nc.vector.tensor_copy(dslice, qp[:])
                    else:
                        nc.scalar.copy(dslice, qp[:])

            qk_chunk(q_sb, wq_sb, 0, 0)
            qk_chunk(k_sb, wk_sb, 0, 1)
            qk_chunk(q_sb, wq_sb, 1, 0)
            qk_chunk(k_sb, wk_sb, 1, 1)

            # V^T zero fill + ones columns on the otherwise-idle gpsimd
            nc.gpsimd.memset(vt_sb[:], 0.0)
            ones_ap = vt_sb[:].rearrange(
                "q (p h e) -> q p h e", p=MT, h=H, e=32)[:, :, :, 16:17]
            nc.gpsimd.memset(ones_ap, 1.0)

            # V^T: 16 key tiles, two 512-col psum ring slots of 8 tiles each
            for half in range(2):
                vp = ps_a.tile([C2, 512], F32, tag="sa", name="vp")
                for p in range(8):
                    pk = 8 * half + p
                    nc.tensor.matmul(vp[:, C * p:C * (p + 1)],
                                     lhsT=x_sb[:, 128 * pk:128 * (pk + 1)],
                                     rhs=wv_sb[:])
                vt_dst = vt_sb[:, 1024 * half:1024 * (half + 1)].rearrange(
                    "q (p h e) -> q p h e", p=8, h=H, e=32)[:, :, :, 0:16]
                vt_src = vp[:].rearrange(
                    "q (p h d) -> q p h d", p=8, h=H, d=D)
                nc.vector.tensor_copy(vt_dst, vt_src)

            def epilogue(c, pv):
                """Normalize chunk-c attention output, project, evac + stats."""
                # evacuate pv to SBUF (frees the psum slot for the next chunk)
                pvs = ep.tile([C2, NC], F32, tag="pvs")
                nc.vector.tensor_copy(pvs[:], pv[:])
                # denominators pvs[32h+16, :] -> DRAM [4, NC] -> compact
                # [128, 32]; 1/d = exp(-ln d) on ScalarE; back out + bcast
                den_d = dram.tile([H, NC], F32, tag="den_d")
                for h in range(H):
                    nc.sync.dma_start(den_d[h:h + 1, :],
                                      pvs[32 * h + 16:32 * h + 17, :])
                den_c = ep.tile([C2, NC // 32], F32, tag="den_c")
                nc.sync.dma_start(
                    den_c[:],
                    den_d[:].rearrange("h (p q) -> (h p) q", p=32, q=32))
                lnd = ep.tile([C2, NC // 32], F32, tag="lnd")
                nc.scalar.activation(lnd[:], den_c[:], Act.Ln)
                rcp = ep.tile([C2, NC // 32], F32, tag="rcp")
                nc.scalar.activation(rcp[:], lnd[:], Act.Exp, scale=-1.0)
                rec_d = dram.tile([H, NC], F32, tag="rec_d")
                nc.sync.dma_start(
                    rec_d[:].rearrange("h (p q) -> (h p) q", p=32, q=32),
                    rcp[:])
                rbc = ep.tile([C2, NC], F32, tag="rbc")
                for h in range(H):
                    nc.sync.dma_start(
                        rbc[32 * h:32 * h + 32, :],
                        rec_d[h:h + 1, :].partition_broadcast(32))
                on = ep.tile([C2, NC], BF16, tag="on")
                nc.vector.tensor_mul(on[:], pvs[:], rbc[:])
                # projection into s-ring slots; chunk c rows 64c..64c+64
                r = slice(64 * c, 64 * c + 64)
                for j in range(NC // 512):
                    yp = ps_a.tile([C2, 512], F32, tag="sa", name="yp")
                    nc.tensor.matmul(yp[r, :],
                                     lhsT=wp_sb[:], rhs=on[:, 512 * j:512 * (j + 1)],
                                     tile_position=(0, 64 * c))
                    nc.vector.tensor_scalar(y_sb[r, 512 * j:512 * (j + 1)],
                                            yp[r, :], 1.0, 0.0,
                                            op0=Alu.mult, op1=Alu.add,
                                            accum_out=stats[r, j:j + 1])
                    ysq = ep.tile([C2, 512], F32, tag="ysq")
                    nc.vector.scalar_tensor_tensor(
                        ysq[r, :], y_sb[r, 512 * j:512 * (j + 1)], 0.0,
                        y_sb[r, 512 * j:512 * (j + 1)], op0=Alu.add,
                        op1=Alu.mult, accum_out=stats[r, 2 + j:3 + j])

            # ---- attention: per (chunk, key-tile, j-half): 4 S matmuls into
            # the 5-deep psum ring (distinct PE row-groups -> concurrent),
            # exp on ScalarE (heads 0,2) / VectorE (heads 1,3 bit-trick),
            # then 4 col-tiled PV matmuls accumulate into the pv slot.  The
            # PV group lags one (t,j) step so the PE emits the NEXT step's
            # score matmuls before the PV that waits on this step's exps —
            # the exp engines then never wait on a fresh score tile.
            for c in range(NCH):
                pv = ps_pv.tile([C2, NC], F32, tag="pv", name="pv")
                prev_pts = None

                def flush_pv(pv=pv):
                    nonlocal prev_pts
                    if prev_pts is None:
                        return
                    for pt, t, j, h in prev_pts:
                        nc.tensor.matmul(
                            pv[32 * h:32 * h + 32, 512 * j:512 * (j + 1)],
                            lhsT=vt_sb[:, 128 * t + 32 * h: 128 * t + 32 * h + 32],
                            rhs=pt[:],
                            start=(t == 0), stop=(t == MT - 1),
                            skip_group_check=True,
                            tile_position=(0, 32 * h))
                    prev_pts = None

                for t in range(MT):
                    for j in range(NC // 512):
                        sps = []
                        for h in range(H):
                            if h in ACT_HEADS:
                                sp = ps_a.tile([C2, 512], F32, tag="sa",
                                               name="spa")
                            else:
                                sp = ps_d.tile([C2, 512], F32, tag="sd",
                                               name="spd")
                            nc.tensor.matmul(
                                sp[:],
                                lhsT=k_sb[32 * h:32 * h + 16, 128 * t:128 * (t + 1)],
                                rhs=q_sb[32 * h:32 * h + 16,
                                         NC * c + 512 * j: NC * c + 512 * (j + 1)],
                                tile_position=(32 * h, 0))
                            sps.append((sp, h))
                        pts = []
                        for sp, h in sps:
                            if h in ACT_HEADS:
                                pt = pp.tile([C2, 512], BF16, tag="p")
                                nc.scalar.activation(pt[:], sp[:], Act.Exp,
                                                     scale=SCALE)
                                pts.append((pt[:], t, j, h))
                            else:
                                pti = fp.tile([C2, 512], I16, tag="pf")
                                nc.vector.tensor_scalar(pti[:], sp[:], FA, FB,
                                                        op0=Alu.mult,
                                                        op1=Alu.add)
                                pts.append((pti[:].bitcast(BF16), t, j, h))
                        flush_pv()
                        prev_pts = pts
                flush_pv()
                epilogue(c, pv)

            # ---- cross-core reduce of batchnorm stats (prefold j-pairs
            # to [128, 2] so the collective moves 1KB)
            st2 = sb.tile([C2, 2], F32, tag="st2")
            nc.gpsimd.tensor_add(st2[:], stats[:, 0:4:2], stats[:, 1:4:2])
            st_in = dram.tile([C2, 2], F32, tag="st_in")
            st_out = dram.tile([C2, 2], F32, tag="st_out")
            nc.gpsimd.dma_start(st_in[:], st2[:])
            nc.gpsimd.collective_compute(
                "AllReduce", Alu.add,
                replica_groups=[list(range(N_CORES))],
                ins=[st_in.opt()], outs=[st_out.opt()])
            # fold the two channel halves and replicate to all 128 partitions
            fa = sb.tile([C2, 2], F32, tag="fa")
            fb = sb.tile([C2, 2], F32, tag="fb")
            nc.sync.dma_start(fa[:], st_out[:])
            nc.sync.dma_start(fb[0:C, :], st_out[C:C2, :])
            nc.sync.dma_start(fb[C:C2, :], st_out[0:C, :])
            # ---- finalize, single-engine chain on vector (ln/exp on ScalarE)
            me = sb.tile([C2, 2], F32, tag="me")     # [mean | E x^2]
            nc.vector.tensor_add(me[:], fa[:], fb[:])
            nc.vector.tensor_scalar_mul(me[:], me[:], 1.0 / CNT)
            mean = me[:, 0:1]
            msq = sb.tile([C2, 1], F32, tag="msq")
            nc.vector.tensor_mul(msq[:], me[:, 0:1], me[:, 0:1])
            var = sb.tile([C2, 1], F32, tag="var")
            nc.vector.tensor_sub(var[:], me[:, 1:2], msq[:])
            eps_t = sb.tile([C2, 1], F32, tag="eps")
            nc.gpsimd.memset(eps_t[:], BN_EPS)
            lnv = sb.tile([C2, 1], F32, tag="lnv")
            nc.scalar.activation(lnv[:], var[:], Act.Ln, bias=eps_t[:, 0:1])
            istd = sb.tile([C2, 1], F32, tag="istd")
            nc.scalar.activation(istd[:], lnv[:], Act.Exp, scale=-0.5)
            sc = sb.tile([C2, 1], F32, tag="sc")
            nc.vector.tensor_mul(sc[:], g_sb[:], istd[:])
            msc = sb.tile([C2, 1], F32, tag="msc")
            nc.vector.tensor_scalar(msc[:], mean, sc[:, 0:1], None, op0=Alu.mult)
            sh = sb.tile([C2, 1], F32, tag="sh")
            nc.vector.tensor_sub(sh[:], b_sb[:], msc[:])

            # ---- normalize + LeakyReLU + store (gpsimd + vector split)
            yn = ep.tile([C2, NC], F32, tag="rbc")
            nc.vector.tensor_scalar(yn[:], y_sb[:],
                                    sc[:, 0:1], sh[:, 0:1],
                                    op0=Alu.mult, op1=Alu.add)
            yl = ep.tile([C2, NC], F32, tag="on2")
            nc.vector.scalar_tensor_tensor(yl[:], yn[:], LEAK, yn[:],
                                           op0=Alu.mult, op1=Alu.max)

            nc.sync.dma_start(out_p[:, 0:NC], yl[0:C, :])
            nc.sync.dma_start(out_p[:, NC:N], yl[C:C2, :])

    nc.compile()

    # Post-compile surgery: one activation table set covers Exp+Ln; point the
    # first inserted load at it and drop the rest (loads are inserted after
    # semaphore generation, so they carry no sync state and removal is safe).
    from concourse.hw_specs import get_activation_tables
    tabs = list(get_activation_tables(nc.m.arch).keys())
    nle = tabs.index("natural_log_exp_and_others")
    loads = [(b, i) for b in nc.main_func.blocks for i in b.instructions
             if isinstance(i, mybir.InstLoadActFuncSet)]
    if loads:
        loads[0][1].act_func_set_id = nle
        for b, i in loads[1:]:
            b.instructions.remove(i)
    return nc


_NC_CACHE = None


def _get_nc():
    global _NC_CACHE
    if _NC_CACHE is None:
        _NC_CACHE = build()
    return _NC_CACHE


def _prep_inputs(x_local, x_branch, w_qkv, w_proj, gamma, beta):
    bf16 = ml_dtypes.bfloat16
    x_local = np.asarray(x_local, np.float32)
    x_branch = np.asarray(x_branch, np.float32)
    w_qkv = np.asarray(w_qkv, np.float32)
    w_proj = np.asarray(w_proj, np.float32)
    gamma = np.asarray(gamma, np.float32)
    beta = np.asarray(beta, np.float32)

    X = np.concatenate([x_local, x_branch], axis=1).astype(bf16)  # [B, 128, N]
    WT = w_qkv.T.copy()  # [128, 192]
    wq = np.zeros((C2, 128), np.float32)
    wk = np.zeros((C2, 128), np.float32)
    for h in range(H):
        wq[:, 32 * h:32 * h + D] = WT[:, D * h:D * (h + 1)]
        wk[:, 32 * h:32 * h + D] = WT[:, C + D * h:C + D * (h + 1)]
    wv = WT[:, 2 * C:3 * C]
    wp = np.zeros((C2, C), np.float32)
    for h in range(H):
        wp[32 * h:32 * h + D, :] = w_proj[:, D * h:D * (h + 1)].T
    common = dict(
        wq=wq.astype(bf16), wk=wk.astype(bf16), wv=np.ascontiguousarray(wv).astype(bf16),
        wp=wp.astype(bf16),
        gamma=np.ascontiguousarray(gamma.reshape(C, 1)),
        beta=np.ascontiguousarray(beta.reshape(C, 1)),
    )
    return [dict(x=np.ascontiguousarray(X[b]), **common) for b in range(B)]


def kernel(x_local, x_branch, w_qkv, w_proj, gamma, beta, _trace=False, _tmpdir=None):
    nc = _get_nc()
    in_maps = _prep_inputs(x_local, x_branch, w_qkv, w_proj, gamma, beta)
    res = run_bass_kernel_spmd(nc, in_maps, core_ids=list(range(N_CORES)),
                               trace=_trace, tmpdir=_tmpdir)
    out = np.stack([np.asarray(res.results[i]["out"]) for i in range(N_CORES)])
    if _trace:
        kernel._last_results = res
    return out.astype(np.float32)
